# revision 1
# baseline (speedup 1.0000x reference)
"""Trainium2 Bass kernel for nn_BasicFlow (sparse window attention flow).

Sharding: pure data-parallel over batch B=8 -> one image pair per NeuronCore.
Device computes (per core, in bf16 on the PE):
  - 4x conv3x3 (128->128ch, 96x96) as 9 shifted accumulating matmuls
  - all 8 shift-variant x 144-window correlation matmuls (64x64 per window)
and writes the raw correlation volumes to DRAM. The small softmax/flow/splice/
bilinear tail (~1% of FLOPs) is vectorized numpy on host.
"""

import os

# recover wedged NeuronCores at NRT init (observed transient
# NRT_EXEC_UNIT_UNRECOVERABLE; reset-on-load clears it)
os.environ.setdefault("NEURON_RT_RESET_CORES", "1")

import numpy as np
import ml_dtypes

import concourse.bass as bass
import concourse.bacc as bacc
import concourse.tile as tile
import concourse.mybir as mybir
from concourse import bass_utils

F32 = mybir.dt.float32
BF16 = mybir.dt.bfloat16

B = 8
DIM = 128
H = W = 96
P = 8
UP = 4
SCALE = DIM ** -0.5
S1 = S2 = H // P          # 12 windows per axis
NW = S1 * S2              # 144 windows
NV = 8                    # 4 shift variants x 2 directions

_COMPILED = None


# --------------------------------------------------------------------------
# Device kernel
# --------------------------------------------------------------------------

def _build_device():
    nc = bacc.Bacc("TRN2", target_bir_lowering=False, debug=False, num_devices=8)

    f0_d = nc.dram_tensor("f0", [DIM, H, W], BF16, kind="ExternalInput")
    f2_d = nc.dram_tensor("f2", [DIM, H, W], BF16, kind="ExternalInput")
    wq_d = nc.dram_tensor("wq", [DIM, 9, DIM], BF16, kind="ExternalInput")
    wk_d = nc.dram_tensor("wk", [DIM, 9, DIM], BF16, kind="ExternalInput")
    bq_d = nc.dram_tensor("bq", [DIM, 1], F32, kind="ExternalInput")
    bk_d = nc.dram_tensor("bk", [DIM, 1], F32, kind="ExternalInput")
    # raw correlation volumes, window pairs packed across 128 partitions:
    # [variant*2+dir, par*64+q_pixel, window_pair, k_pixel], window = 2*pair+par
    corr_d = nc.dram_tensor("corr", [NV, 2 * P * P, NW // 2, P * P], BF16,
                            kind="ExternalOutput")

    with tile.TileContext(nc) as tc:
        with (
            tc.tile_pool(name="const", bufs=1) as constp,
            tc.tile_pool(name="big", bufs=5) as bigp,
            tc.tile_pool(name="qk", bufs=1) as qkp,
            tc.tile_pool(name="stage", bufs=10) as stagep,
            tc.tile_pool(name="psum", bufs=8, space="PSUM") as psump,
        ):
            wq_sb = constp.tile([DIM, 9, DIM], BF16, tag="wq")
            wk_sb = constp.tile([DIM, 9, DIM], BF16, tag="wk")
            bq_sb = constp.tile([DIM, 1], F32, tag="bq")
            bk_sb = constp.tile([DIM, 1], F32, tag="bk")
            nc.sync.dma_start(wq_sb[:], wq_d[:])
            nc.sync.dma_start(wk_sb[:], wk_d[:])
            nc.sync.dma_start(bq_sb[:], bq_d[:])
            nc.sync.dma_start(bk_sb[:], bk_d[:])

            q0 = qkp.tile([DIM, H, W], BF16, tag="q0")
            k0 = qkp.tile([DIM, H, W], BF16, tag="k0")
            q2 = qkp.tile([DIM, H, W], BF16, tag="q2")
            k2 = qkp.tile([DIM, H, W], BF16, tag="k2")

            RT = 4                       # output rows per psum tile
            NRT = H // RT

            def conv(dst, fpad, w_sb, b_sb):
                for rt in range(NRT):
                    ps = psump.tile([DIM, RT, W], F32, tag="ps")
                    for t in range(9):
                        dy, dx = divmod(t, 3)
                        rhs = fpad[:, rt * RT + dy: rt * RT + dy + RT,
                                   dx: dx + W]
                        nc.tensor.matmul(ps[:], w_sb[:, t, :], rhs,
                                         start=(t == 0), stop=(t == 8))
                    nc.scalar.activation(
                        dst[:, rt * RT:(rt + 1) * RT, :], ps[:],
                        mybir.ActivationFunctionType.Identity, bias=b_sb[:])

            # Load both padded feature maps upfront (slots shared with the
            # window-major tiles below), then conv in q0,k2,q2,k0 order so
            # the first correlation variant can start mid-conv-phase.
            fpads = []
            for src_d in (f0_d, f2_d):
                fpad = bigp.tile([DIM, H + 2, W + 2], BF16, tag="big")
                # zero only the 1-px border; interior is fully DMA-written
                nc.vector.memset(fpad[:, 0, :], 0.0)
                nc.vector.memset(fpad[:, H + 1, :], 0.0)
                nc.vector.memset(fpad[:, 1:H + 1, 0], 0.0)
                nc.vector.memset(fpad[:, 1:H + 1, W + 1], 0.0)
                # chunked load: first conv row-tiles start before the full
                # feature map lands
                CH = H // 4
                for c in range(4):
                    nc.sync.dma_start(
                        fpad[:, 1 + c * CH:1 + (c + 1) * CH, 1:W + 1],
                        src_d[:, c * CH:(c + 1) * CH, :])
                fpads.append(fpad)
            conv(q0, fpads[0], wq_sb, bq_sb)
            conv(k2, fpads[1], wk_sb, bk_sb)
            conv(q2, fpads[1], wq_sb, bq_sb)
            conv(k0, fpads[0], wk_sb, bk_sb)

            def _boxes(r):
                # (w0, nw, l0, nl) boxes over (window, local) of one axis so
                # that src rows w*8+l+r (mod 96) are contiguous per box
                if r == 0:
                    return [(0, S2, 0, P)]
                return [(0, S2 - 1, 0, P), (S2 - 1, 1, 0, P - r),
                        (S2 - 1, 1, P - r, r)]

            def wm_copy(dst, src, ry, rx, eng):
                # dst[ch, wy*12+wx, ly*8+lx] = src[ch, (wy*8+ly+ry)%96,
                #                                      (wx*8+lx+rx)%96]
                dstv = dst[:].rearrange("p (wy wx) (ly lx) -> p wy wx ly lx",
                                        wx=S2, lx=P)
                for wy0, nwy, ly0, nly in _boxes(ry):
                    for wx0, nwx, lx0, nlx in _boxes(rx):
                        d = dstv[:, wy0:wy0 + nwy, wx0:wx0 + nwx,
                                 ly0:ly0 + nly, lx0:lx0 + nlx]
                        r0 = (wy0 * P + ly0 + ry) % H
                        c0 = (wx0 * P + lx0 + rx) % W
                        s = src[:, r0:r0 + (nwy - 1) * P + nly,
                                c0:c0 + (nwx - 1) * P + nlx]
                        s = s.rearrange("p (wy ly) (wx lx) -> p wy wx ly lx",
                                        ly=nly, lx=nlx)
                        eng.tensor_copy(d, s)

            WG = 16                     # windows per psum bank (fills 2KB)
            for v in range(4):
                ry = 4 if v >= 2 else 0
                rx = 4 if (v % 2) else 0
                for d in range(2):
                    qs_base, ks_base = (q0, k2) if d == 0 else (q2, k0)
                    qs = bigp.tile([DIM, NW, P * P], BF16, tag="big")
                    ks = bigp.tile([DIM, NW, P * P], BF16, tag="big")
                    wm_copy(qs, qs_base, ry, rx, nc.vector)
                    wm_copy(ks, ks_base, ry, rx, nc.gpsimd)
                    vd = v * 2 + d
                    # even window -> PE col-groups 0-1 (psum partitions 0-63),
                    # odd window -> col-groups 2-3 (64-127); pairs run
                    # concurrently in the array
                    for wg in range(NW // WG):
                        ps = psump.tile([2 * P * P, WG // 2, P * P], F32,
                                        tag="ps")
                        sb = stagep.tile([2 * P * P, WG // 2, P * P], BF16,
                                         tag="corrsb")
                        for wi in range(WG // 2):
                            w = wg * WG + 2 * wi
                            nc.tensor.matmul(ps[0:64, wi, :], qs[:, w, :],
                                             ks[:, w, :], start=True,
                                             stop=True, tile_position=(0, 0))
                            nc.tensor.matmul(ps[64:128, wi, :],
                                             qs[:, w + 1, :], ks[:, w + 1, :],
                                             start=True, stop=True,
                                             tile_position=(0, 64))
                        if wg % 3 == 1:
                            nc.vector.tensor_copy(sb[:], ps[:])
                        else:
                            nc.scalar.copy(sb[:], ps[:])
                        nc.sync.dma_start(
                            corr_d[vd, :, wg * (WG // 2):(wg + 1) * (WG // 2),
                                   :], sb[:])

    nc.compile()
    return nc


def _run_device(feat0, feat2, wq, bq, wk, bk):
    global _COMPILED
    if _COMPILED is None:
        _COMPILED = _build_device()
    nc = _COMPILED

    bf = ml_dtypes.bfloat16
    wqT = np.ascontiguousarray(
        wq.astype(np.float32).transpose(1, 2, 3, 0).reshape(DIM, 9, DIM)
    ).astype(bf)
    wkT = np.ascontiguousarray(
        wk.astype(np.float32).transpose(1, 2, 3, 0).reshape(DIM, 9, DIM)
    ).astype(bf)
    bqc = np.ascontiguousarray(bq.astype(np.float32).reshape(DIM, 1))
    bkc = np.ascontiguousarray(bk.astype(np.float32).reshape(DIM, 1))

    in_maps = []
    for b in range(B):
        in_maps.append({
            "f0": np.ascontiguousarray(feat0[b]).astype(bf),
            "f2": np.ascontiguousarray(feat2[b]).astype(bf),
            "wq": wqT, "wk": wkT, "bq": bqc, "bk": bkc,
        })
    import os
    trace = bool(int(os.environ.get("BASSFLOW_TRACE", "0")))
    res = bass_utils.run_bass_kernel_spmd(nc, in_maps, core_ids=list(range(B)),
                                          trace=trace)
    if trace:
        print(f"HW exec time: {res.exec_time_ns} ns "
              f"(mean {res.mean_exec_time_ns})")
        if res.instructions_and_trace:
            print("trace path:", res.instructions_and_trace[1])
    corr = np.stack([res.results[b]["corr"] for b in range(B)])
    # [B, NV, par*64+q, pair, k] -> [B, NV, win=2*pair+par, q, k]
    corr = corr.reshape(B, NV, 2, P * P, NW // 2, P * P)
    corr = corr.transpose(0, 1, 4, 2, 3, 5).reshape(B, NV, NW, P * P, P * P)
    return corr.astype(np.float32)


# --------------------------------------------------------------------------
# Host tail: bias/mask + softmax flow pipeline + splice + bilinear upsample
# (numpy port of the reference; ~1% of total FLOPs)
# --------------------------------------------------------------------------

def _bias_index():
    coords = np.stack(np.meshgrid(np.arange(P), np.arange(P),
                                  indexing='ij')).reshape(2, -1)
    rel = (coords[:, :, None] - coords[:, None, :]).transpose(1, 2, 0).copy()
    rel[..., 0] += P - 1
    rel[..., 1] += P - 1
    rel[..., 0] *= 2 * P - 1
    return rel.sum(-1).reshape(-1)


def _pos():
    r = np.arange(P, dtype=np.float32)
    yy, xx = np.meshgrid(r, r, indexing='ij')
    return np.stack([xx, yy])[None].reshape(1, 2, P * P)


def _make_mask(Hp, Wp, sh, sw):
    m = np.zeros((Hp, Wp))
    hs = ((slice(0, -sh * 2), slice(-sh * 2, -sh), slice(-sh, None))
          if sh else (slice(None),))
    ws = ((slice(0, -sw * 2), slice(-sw * 2, -sw), slice(-sw, None))
          if sw else (slice(None),))
    cnt = 0
    for a in hs:
        for b in ws:
            m[a, b] = cnt
            cnt += 1
    win = m.reshape(Hp // P, P, Wp // P, P).transpose(0, 2, 1, 3).reshape(-1, P * P)
    d = win[:, None, :] - win[:, :, None]
    return np.where(d != 0, -10000.0, 0.0).astype(np.float32)


def _softmax(x, axis):
    m = np.max(x, axis=axis, keepdims=True)
    e = np.exp(x - m)
    return e / np.sum(e, axis=axis, keepdims=True)


_MID_IDX = None


def _mid_gather():
    """c[b, (j,k), (h2,w2)] = corr[b, (j+3-h2, k+3-w2), (h2,w2)] (0 if invalid)."""
    global _MID_IDX
    if _MID_IDX is None:
        j, k, h2, w2 = np.meshgrid(np.arange(9), np.arange(9), np.arange(P),
                                   np.arange(P), indexing='ij')
        qy = j + 3 - h2
        qx = k + 3 - w2
        valid = (qy >= 0) & (qy < P) & (qx >= 0) & (qx < P)
        qidx = np.clip(qy, 0, P - 1) * P + np.clip(qx, 0, P - 1)
        kidx = h2 * P + w2
        _MID_IDX = (qidx.reshape(81, 64), kidx.reshape(81, 64),
                    valid.reshape(81, 64))
    return _MID_IDX


def _flow_mid(corr, pos):
    bw = corr.shape[0]
    qidx, kidx, valid = _mid_gather()
    c = corr[:, qidx, kidx] * valid[None]          # (bw, 81, 64)
    n = P + 1
    r = np.arange(0.0, P - 0.5, 0.5)
    yy, xx = np.meshgrid(r, r, indexing='ij')
    CH = P // 2 - 1
    base = np.stack([xx, yy])[None][:, :, CH:2 * P - 1 - CH, CH:2 * P - 1 - CH]
    base = base.reshape(1, 2, n * n).astype(np.float32)
    flow = pos[:, :, None, :] - base[:, :, :, None]          # (1,2,81,64)
    smax = _softmax(c, axis=2)
    fl = np.einsum('bmk,cmk->bcm', smax, flow[0]).reshape(bw, 2, n, n)
    cr = np.sum(c * smax, axis=2).reshape(bw, 1, n, n)
    corr4 = np.concatenate([cr[:, :, :-1, :-1], cr[:, :, :-1, 1:],
                            cr[:, :, 1:, :-1], cr[:, :, 1:, 1:]], axis=1)
    flow4 = np.concatenate([fl[:, :, :-1, :-1], fl[:, :, :-1, 1:],
                            fl[:, :, 1:, :-1], fl[:, :, 1:, 1:]], axis=1)
    corr4 = corr4.transpose(0, 2, 3, 1).reshape(bw, P * P, 4)
    flow4 = flow4.reshape(bw, 4, 2, P, P).transpose(0, 2, 3, 4, 1)
    flow4 = flow4.reshape(bw, 2, P * P, 4) * 2
    smax2 = _softmax(corr4, axis=2)
    out = np.sum(flow4 * smax2[:, None], axis=3)
    return out.reshape(bw, 2, P, P).astype(np.float32)


def _flow_bsd(corr, pos):
    cut = P // 4
    bw = corr.shape[0]
    c = corr.reshape(bw, P, P, P * P)[:, cut:P - cut, cut:P - cut, :]
    L = (P - 2 * cut) ** 2
    c = c.reshape(bw, L, P * P)
    base = _pos().reshape(1, 2, P, P)[:, :, cut:P - cut, cut:P - cut]
    base = base.reshape(1, 2, L)
    flow = pos[:, :, None, :] - base[:, :, :, None]
    smax = _softmax(c, axis=2)
    out = np.einsum('blk,clk->bcl', smax, flow[0])
    return out.reshape(bw, 2, P - 2 * cut, P - 2 * cut).astype(np.float32)


def _splice(f00, f01, f10, f11, factor, Ho, Wo):
    f = np.concatenate([np.concatenate([f00, f01], axis=3),
                        np.concatenate([f10, f11], axis=3)], axis=2)
    bs, kk, hh, ww = f.shape
    b = bs // (S1 * S2)
    f = f.reshape(b, S1, S2, kk, hh, ww).transpose(0, 3, 1, 4, 2, 5)
    f = f.reshape(b, kk, S1 * hh, S2 * ww)
    sft = (P // 4) * factor
    f = np.roll(f, (sft, sft), axis=(2, 3))
    return f[:, :, :Ho * factor, :Wo * factor]


def _resize_mat(in_size, out_size):
    scale = out_size / in_size
    sample = (np.arange(out_size) + 0.5) / scale - 0.5
    x = np.abs(sample[None, :] - np.arange(in_size)[:, None])
    w = np.maximum(0.0, 1.0 - x)
    tot = w.sum(0, keepdims=True)
    return (w / np.where(tot == 0, 1.0, tot)).astype(np.float32)


def _up(x, f):
    b, c, h, w = x.shape
    My = _resize_mat(h, h * f)
    Mx = _resize_mat(w, w * f)
    y = np.einsum('bchw,hH->bcHw', x, My)
    y = np.einsum('bcHw,wW->bcHW', y, Mx)
    return (y * f).astype(np.float32)


def _host_flow(corr_raw, bias_table):
    """corr_raw: (B, NV, NW, 64, 64) raw q.k^T dot products."""
    bias = bias_table.astype(np.float32)[_bias_index()].reshape(
        P * P, P * P, 1).transpose(2, 0, 1)          # (1,64,64)
    pos = _pos()
    masks = {}
    for v, (sh, sw) in enumerate(((0, 0), (0, 4), (4, 0), (4, 4))):
        masks[v] = _make_mask(H, W, sh, sw) if (sh or sw) else None

    f1 = {}
    f0 = {}
    for v in range(4):
        for d in range(2):
            c = corr_raw[:, v * 2 + d].reshape(B * NW, 64, 64) * SCALE + bias
            if masks[v] is not None:
                c = (c.reshape(B, NW, 64, 64) + masks[v][None]).reshape(
                    B * NW, 64, 64)
            f1[(v, d)] = _flow_mid(c, pos)
            f0[(v, d)] = _flow_bsd(c, pos)

    # direction 0: (q0,k2) -> flow12 (mid), flow02 (bsd)
    # direction 1: (q2,k0) -> flow10 (mid), flow20 (bsd)
    flow12 = _splice(f1[(0, 0)], f1[(1, 0)], f1[(2, 0)], f1[(3, 0)], 2, H, W)
    flow02 = _splice(f0[(0, 0)], f0[(1, 0)], f0[(2, 0)], f0[(3, 0)], 1, H, W)
    flow10 = _splice(f1[(0, 1)], f1[(1, 1)], f1[(2, 1)], f1[(3, 1)], 2, H, W)
    flow20 = _splice(f0[(0, 1)], f0[(1, 1)], f0[(2, 1)], f0[(3, 1)], 1, H, W)
    fh, ff = UP // 2, UP
    return (_up(flow10, fh), _up(flow12, fh), _up(flow02, ff), _up(flow20, ff))


def kernel(feat0, feat2, wq, bq, wk, bk, bias_table):
    corr_raw = _run_device(np.asarray(feat0), np.asarray(feat2),
                           np.asarray(wq), np.asarray(bq),
                           np.asarray(wk), np.asarray(bk))
    return _host_flow(corr_raw, np.asarray(bias_table))

